# revision 1
# baseline (speedup 1.0000x reference)
"""Barnes-Wall (BW16) lattice quantizer kernel for Trainium2, 8-core data-parallel.

Algorithm (validated bit-exact vs the jax reference):
  x = x_in / a   (correctly-rounded via Dekker-product division: p = x*rh plus
                  exact product error + x*rl correction, rh+rl ~ 1/a in double)
  For each of 32 codebook rows c: v = x - c, g = 2*round(v/2) (RNE, via the
  +1.5*2^24 magic-number trick which rounds v to the nearest even integer),
  eneg = g - v (= X - x, exact), D0 = sum(eneg^2), P2 = sum(g) (exact),
  M = max|eneg|.  Parity (sum f odd) is derived from P2/4 vs its RNE rounding.
  Parity-odd candidates pay a flip penalty: D ~ 4*D0 + odd*(16-16M).
  Winner k = first argmin; its X/eneg/parity are extracted with a one-hot
  masked reduce; the parity flip is applied at the first argmax|eneg|
  coordinate with direction -sign(eneg); y = X' * a.
"""
import sys

sys.path.insert(0, "/opt/trn_rl_repo")
import contextlib

import numpy as np

import concourse.bass as bass
import concourse.bacc as bacc
import concourse.mybir as mybir
import concourse.tile as tile

f32 = np.float32
MAGIC = float(f32(1.5 * 2.0**24))   # round-to-even-integer magic
MAGIC1 = float(f32(1.5 * 2.0**23))  # round-to-integer magic (parity)

dt = mybir.dt
Alu = mybir.AluOpType
Act = mybir.ActivationFunctionType
AX = mybir.AxisListType

N_CORES = 8
R = 4  # row blocks of 128 per iteration


def _bcast(ap, pattern):
    return bass.AP(tensor=ap.tensor, offset=ap.offset, ap=[ap.ap[0]] + pattern)


def _div_consts(a_val):
    """rh + rl ~ 1/a (double-float), rhh + rhl = Veltkamp split of rh."""
    r64 = 1.0 / np.float64(f32(a_val))
    rh = f32(r64)
    rl = f32(r64 - np.float64(rh))
    c_ = f32(rh * f32(4097.0))
    rhh = f32(c_ - f32(c_ - rh))
    rhl = f32(rh - rhh)
    return float(rh), float(rl), float(rhh), float(rhl)


def _build(rows, a_val):
    nc = bacc.Bacc("TRN2", target_bir_lowering=False)
    x_d = nc.dram_tensor("x", [rows, 16], dt.float32, kind="ExternalInput")
    cb_d = nc.dram_tensor("cb", [512], dt.float32, kind="ExternalInput")
    i32_d = nc.dram_tensor("i32", [32], dt.float32, kind="ExternalInput")
    i16_d = nc.dram_tensor("i16", [16], dt.float32, kind="ExternalInput")
    y_d = nc.dram_tensor("y", [rows, 16], dt.float32, kind="ExternalOutput")

    rh, rl, rhh, rhl = _div_consts(a_val)

    n_iters = rows // (128 * R)
    assert n_iters * 128 * R == rows

    with tile.TileContext(nc) as tc:
        with contextlib.ExitStack() as ctx:
            singles = ctx.enter_context(tc.tile_pool(name="singles", bufs=1))

            cb_t = singles.tile([128, 512], dt.float32)
            nc.sync.dma_start(out=cb_t, in_=bass.AP(tensor=cb_d, offset=0, ap=[[0, 128], [1, 512]]))
            i32_t = singles.tile([128, 32], dt.float32)
            nc.sync.dma_start(out=i32_t, in_=bass.AP(tensor=i32_d, offset=0, ap=[[0, 128], [1, 32]]))
            i16_t = singles.tile([128, 16], dt.float32)
            nc.sync.dma_start(out=i16_t, in_=bass.AP(tensor=i16_d, offset=0, ap=[[0, 128], [1, 16]]))

            # prologue: xs_all = x / a for the whole shard (Dekker, correctly rounded)
            nrb = rows // 128
            xs_all = singles.tile([128, nrb, 16], dt.float32)
            dkpool_cm = tc.tile_pool(name="dk", bufs=1)
            dkpool = dkpool_cm.__enter__()
            x_all = dkpool.tile([128, nrb, 16], dt.float32)
            nc.sync.dma_start(out=x_all, in_=bass.AP(tensor=x_d, offset=0, ap=[[16, 128], [128 * 16, nrb], [1, 16]]))
            dkA = dkpool.tile([128, nrb, 16], dt.float32)
            dkB = dkpool.tile([128, nrb, 16], dt.float32)
            dkC2 = dkpool.tile([128, nrb, 16], dt.float32)
            dkD = dkpool.tile([128, nrb, 16], dt.float32)
            nc.vector.tensor_scalar(out=dkA, in0=x_all, scalar1=4097.0, scalar2=None, op0=Alu.mult)      # c
            nc.vector.tensor_tensor(out=dkB, in0=dkA, in1=x_all, op=Alu.subtract)                        # u = c - x
            nc.vector.tensor_tensor(out=dkA, in0=dkA, in1=dkB, op=Alu.subtract)                          # xh = c - u
            nc.vector.tensor_tensor(out=dkB, in0=x_all, in1=dkA, op=Alu.subtract)                        # xl
            nc.vector.tensor_scalar(out=dkC2, in0=x_all, scalar1=rh, scalar2=None, op0=Alu.mult)         # p
            nc.vector.scalar_tensor_tensor(out=dkD, in0=dkA, scalar=rhh, in1=dkC2, op0=Alu.mult, op1=Alu.subtract)
            nc.vector.scalar_tensor_tensor(out=dkD, in0=dkA, scalar=rhl, in1=dkD, op0=Alu.mult, op1=Alu.add)
            nc.vector.scalar_tensor_tensor(out=dkD, in0=dkB, scalar=rhh, in1=dkD, op0=Alu.mult, op1=Alu.add)
            nc.vector.scalar_tensor_tensor(out=dkD, in0=dkB, scalar=rhl, in1=dkD, op0=Alu.mult, op1=Alu.add)
            nc.vector.scalar_tensor_tensor(out=dkD, in0=x_all, scalar=rl, in1=dkD, op0=Alu.mult, op1=Alu.add)
            nc.vector.tensor_tensor(out=xs_all, in0=dkC2, in1=dkD, op=Alu.add)
            dkpool_cm.__exit__(None, None, None)
            work = ctx.enter_context(tc.tile_pool(name="work", bufs=4))

            for it in range(n_iters):
                row0 = it * 128 * R
                # --- v = xs - c ---
                v_t = work.tile([128, R, 32, 16], dt.float32)
                xs_sl = xs_all[:, it * R:(it + 1) * R, :]
                xs_b = bass.AP(tensor=xs_sl.tensor, offset=xs_sl.offset, ap=[xs_sl.ap[0], [16, R], [0, 32], [1, 16]])
                cb_b = _bcast(cb_t, [[0, R], [16, 32], [1, 16]])
                nc.vector.tensor_tensor(out=v_t, in0=xs_b, in1=cb_b, op=Alu.subtract)

                # t = v + MAGIC (ACT); g = t - MAGIC (ACT)
                t_t = work.tile([128, R, 32, 16], dt.float32)
                nc.scalar.activation(out=t_t, in_=v_t, func=Act.Copy, bias=MAGIC, scale=1.0)
                g_t = work.tile([128, R, 32, 16], dt.float32)
                nc.scalar.activation(out=g_t, in_=t_t, func=Act.Copy, bias=-MAGIC, scale=1.0)

                # eneg = g - v   (exact via Sterbenz; = X - x); overwrites v in place
                e_t = v_t
                nc.vector.tensor_tensor(out=e_t, in0=g_t, in1=v_t, op=Alu.subtract)

                # sq = eneg^2 (ACT); overwrites t in place
                sq_t = t_t
                nc.scalar.activation(out=sq_t, in_=e_t, func=Act.Square, scale=1.0)

                # per-candidate reductions
                D0 = work.tile([128, R, 32], dt.float32)
                nc.vector.tensor_reduce(out=D0, in_=sq_t, axis=AX.X, op=Alu.add)
                P2 = work.tile([128, R, 32], dt.float32)
                nc.vector.tensor_reduce(out=P2, in_=g_t, axis=AX.X, op=Alu.add)
                M = work.tile([128, R, 32], dt.float32)
                nc.vector.tensor_reduce(out=M, in_=e_t, axis=AX.X, op=Alu.max, apply_absolute_value=True)

                # w = g + c (candidate points X); overwrites g in place
                w_t = g_t
                nc.gpsimd.tensor_tensor(out=w_t, in0=g_t, in1=cb_b, op=Alu.add)


                # parity: h = P2/4; odd <=> h is an odd multiple of 0.5
                h_t = P2  # in-place: P2 dead after h
                nc.vector.tensor_scalar(out=h_t, in0=P2, scalar1=0.25, scalar2=None, op0=Alu.mult)
                th_t = work.tile([128, R, 32], dt.float32)
                nc.scalar.activation(out=th_t, in_=h_t, func=Act.Copy, bias=MAGIC1, scale=1.0)
                hr_t = work.tile([128, R, 32], dt.float32)
                nc.scalar.activation(out=hr_t, in_=th_t, func=Act.Copy, bias=-MAGIC1, scale=1.0)
                dp_t = hr_t  # in-place
                nc.vector.tensor_tensor(out=dp_t, in0=h_t, in1=hr_t, op=Alu.subtract)
                o2_t = work.tile([128, R, 32], dt.float32)  # 0.25 if odd else 0
                nc.scalar.activation(out=o2_t, in_=dp_t, func=Act.Square, scale=1.0)

                # Dq = 4*D0 + (64 - 64*M) * o2
                W64 = work.tile([128, R, 32], dt.float32)
                nc.vector.tensor_scalar(out=W64, in0=M, scalar1=-64.0, scalar2=64.0, op0=Alu.mult, op1=Alu.add)
                pen = W64  # in-place
                nc.vector.tensor_tensor(out=pen, in0=W64, in1=o2_t, op=Alu.mult)
                Dq = D0  # in-place
                nc.vector.scalar_tensor_tensor(out=Dq, in0=D0, scalar=4.0, in1=pen, op0=Alu.mult, op1=Alu.add)

                # first argmin -> one-hot
                Dmin = work.tile([128, R], dt.float32)
                nc.vector.tensor_reduce(out=Dmin, in_=Dq, axis=AX.X, op=Alu.min)
                eq = work.tile([128, R, 32], dt.float32)
                nc.vector.tensor_tensor(out=eq, in0=Dq, in1=_bcast(Dmin, [[1, R], [0, 32]]), op=Alu.is_equal)
                m1 = eq  # in-place
                nc.vector.tensor_tensor(out=m1, in0=eq, in1=_bcast(i32_t, [[0, R], [1, 32]]), op=Alu.mult)
                km = work.tile([128, R], dt.float32)
                nc.vector.tensor_reduce(out=km, in_=m1, axis=AX.X, op=Alu.min)
                onehot = work.tile([128, R, 32], dt.float32)
                nc.vector.tensor_tensor(
                    out=onehot, in0=_bcast(i32_t, [[0, R], [1, 32]]), in1=_bcast(km, [[1, R], [0, 32]]), op=Alu.is_equal
                )

                # masked selects (transposed write then grouped reduce over k)
                oh_b = _bcast(onehot, [[32, R], [1, 32], [0, 16]])

                wT = work.tile([128, R, 16, 32], dt.float32)
                wT_w = bass.AP(tensor=wT.tensor, offset=wT.offset, ap=[wT.ap[0], [512, R], [1, 32], [32, 16]])
                nc.gpsimd.tensor_tensor(out=wT_w, in0=w_t, in1=oh_b, op=Alu.mult)
                wsel = work.tile([128, R, 16], dt.float32)
                nc.vector.tensor_reduce(out=wsel, in_=wT, axis=AX.X, op=Alu.add)

                eT = work.tile([128, R, 16, 32], dt.float32)
                eT_w = bass.AP(tensor=eT.tensor, offset=eT.offset, ap=[eT.ap[0], [512, R], [1, 32], [32, 16]])
                nc.gpsimd.tensor_tensor(out=eT_w, in0=e_t, in1=oh_b, op=Alu.mult)
                esel = work.tile([128, R, 16], dt.float32)
                nc.vector.tensor_reduce(out=esel, in_=eT, axis=AX.X, op=Alu.add)

                o2m = o2_t  # in-place (pen already consumed o2)
                nc.gpsimd.tensor_tensor(out=o2m, in0=o2_t, in1=onehot, op=Alu.mult)
                o2sel = work.tile([128, R], dt.float32)  # 0.25 if odd else 0
                nc.vector.tensor_reduce(out=o2sel, in_=o2m, axis=AX.X, op=Alu.add)

                # parity flip at first argmax|eneg|
                ae = work.tile([128, R, 16], dt.float32)
                nc.scalar.activation(out=ae, in_=esel, func=Act.Abs, scale=1.0)
                M16 = work.tile([128, R], dt.float32)
                nc.vector.tensor_reduce(out=M16, in_=ae, axis=AX.X, op=Alu.max)
                meq = work.tile([128, R, 16], dt.float32)
                nc.vector.tensor_tensor(out=meq, in0=ae, in1=_bcast(M16, [[1, R], [0, 16]]), op=Alu.is_equal)
                m2 = meq  # in-place
                nc.vector.tensor_tensor(out=m2, in0=meq, in1=_bcast(i16_t, [[0, R], [1, 16]]), op=Alu.mult)
                jm = work.tile([128, R], dt.float32)
                nc.vector.tensor_reduce(out=jm, in_=m2, axis=AX.X, op=Alu.min)
                mask1 = work.tile([128, R, 16], dt.float32)
                nc.vector.tensor_tensor(
                    out=mask1, in0=_bcast(i16_t, [[0, R], [1, 16]]), in1=_bcast(jm, [[1, R], [0, 16]]), op=Alu.is_equal
                )
                sgn = work.tile([128, R, 16], dt.float32)
                nc.scalar.activation(out=sgn, in_=esel, func=Act.Sign, scale=1.0)
                u1 = mask1  # in-place
                nc.vector.tensor_tensor(out=u1, in0=mask1, in1=sgn, op=Alu.mult)
                ohalf = work.tile([128, R], dt.float32)  # -2 if odd else 0
                nc.vector.tensor_scalar(out=ohalf, in0=o2sel, scalar1=-8.0, scalar2=None, op0=Alu.mult)
                u2 = u1  # in-place
                nc.vector.tensor_tensor(out=u2, in0=u1, in1=_bcast(ohalf, [[1, R], [0, 16]]), op=Alu.mult)
                Xf = wsel  # in-place
                nc.vector.tensor_tensor(out=Xf, in0=wsel, in1=u2, op=Alu.add)
                y_t = Xf  # in-place
                nc.vector.tensor_scalar(out=y_t, in0=Xf, scalar1=float(f32(a_val)), scalar2=None, op0=Alu.mult)

                nc.sync.dma_start(
                    out=bass.AP(tensor=y_d, offset=row0 * 16, ap=[[16, 128], [128 * 16, R], [1, 16]]),
                    in_=y_t,
                )
    nc.finalize()
    return nc


_CACHE = {}


def _get_nc(rows, a_val):
    key = (rows, a_val)
    if key not in _CACHE:
        _CACHE[key] = _build(rows, a_val)
    return _CACHE[key]


def kernel(x_in, C_rep, a):
    from concourse.bass_utils import run_bass_kernel_spmd

    x = np.ascontiguousarray(np.asarray(x_in, dtype=np.float32))
    C = np.asarray(C_rep, dtype=np.float32)
    a_val = float(np.asarray(a).reshape(-1)[0])
    B = x.shape[0]
    rows = B // N_CORES
    assert rows * N_CORES == B

    nc = _get_nc(rows, a_val)

    cb_np = C.reshape(-1).astype(np.float32)
    i32_np = (np.arange(32) - 64).astype(np.float32)
    i16_np = (np.arange(16) - 32).astype(np.float32)
    shards = x.reshape(N_CORES, rows, 16)
    in_maps = [
        {"x": shards[i], "cb": cb_np, "i32": i32_np, "i16": i16_np}
        for i in range(N_CORES)
    ]
    res = run_bass_kernel_spmd(nc, in_maps, core_ids=list(range(N_CORES)))
    y = np.concatenate([res.results[i]["y"] for i in range(N_CORES)], axis=0)
    return y.astype(np.float32)


if __name__ == "__main__":
    rng = np.random.default_rng(0)
    x = rng.standard_normal((262144, 16), dtype=np.float32)
    C = rng.integers(0, 5, size=(32, 16)).astype(np.float32)
    a = np.array([0.59460354], dtype=np.float32)
    y = kernel(x, C, a)
    print("ok", y.shape, y.dtype)



# revision 9
# speedup vs baseline: 2.4820x; 2.4820x over previous
"""Barnes-Wall (BW16) lattice quantizer for Trainium2, 8-core data-parallel.

Algorithm (validated bit-exact vs the jax reference in numpy):
  The 32 codewords C differ per-dimension only through parity (their mod-2
  patterns are the RM(1,4) codewords).  So per row only TWO rounding chains
  are needed: r0 = nearest even integer vector to x' = x/a, r1+1 = nearest odd
  integer vector, with offsets e0 = r0-x', e1 = (r1+1)-x' (e1-e0 = dr = +-1
  exactly).  Every per-candidate quantity is then a parity-masked sum:
    D0_k = sum(s0) + sum(par_k * q)       (q = e1^2-e0^2, s0 = e0^2)
    P2_k = sum(r0) + sum(par_k * dr) - SC_k
  computed on the idle PE as block-diagonal matmuls in a transposed layout
  (partitions = 8 batch-sub x 16 dims).  The flip penalty needs
  M_k = max_d(par ? 1-|e0| : |e0|) = 0.5 + max_d(sp_k * u5), u5 = |e0|-0.5,
  done per complement-pair (even codewords only, max and min reduces) in row
  layout after PE transposes.  Winner selection packs (k, odd, parity-bits)
  into one fp32 constant per candidate so a single min-reduce recovers
  everything; parity bits are unpacked with exact fp32 mod arithmetic.
"""
import sys

sys.path.insert(0, "/opt/trn_rl_repo")
import contextlib

import numpy as np

import concourse.bass as bass
import concourse.bacc as bacc
import concourse.mybir as mybir
import concourse.tile as tile
from concourse.masks import make_identity

f32 = np.float32
MAGIC = float(f32(1.5 * 2.0**24))   # round-to-even-integer magic
MAGIC1 = float(f32(1.5 * 2.0**23))  # round-to-integer magic (parity)

dt = mybir.dt
Alu = mybir.AluOpType
Act = mybir.ActivationFunctionType
AX = mybir.AxisListType

N_CORES = 8
F = 512  # b_hi columns per chunk


def _div_consts(a_val):
    r64 = 1.0 / np.float64(f32(a_val))
    rh = f32(r64)
    rl = f32(r64 - np.float64(rh))
    return float(rh), float(rl)


def _ap(t, pattern, off=0):
    return bass.AP(tensor=t.tensor, offset=t.offset + off, ap=[t.ap[0]] + pattern)


def _build(rows, a_val):
    nc = bacc.Bacc("TRN2", target_bir_lowering=False)
    x_d = nc.dram_tensor("x", [rows, 16], dt.float32, kind="ExternalInput")
    wm_d = nc.dram_tensor("wm", [3 * 16384], dt.float32, kind="ExternalInput")
    b4_d = nc.dram_tensor("b4", [256], dt.float32, kind="ExternalInput")
    sp_d = nc.dram_tensor("sp", [256], dt.float32, kind="ExternalInput")
    ck_d = nc.dram_tensor("ck", [32], dt.float32, kind="ExternalInput")
    i16_d = nc.dram_tensor("i16", [16], dt.float32, kind="ExternalInput")
    shv_d = nc.dram_tensor("shv", [16], dt.int32, kind="ExternalInput")
    y_d = nc.dram_tensor("y", [rows, 16], dt.float32, kind="ExternalOutput")

    rh, rl = _div_consts(a_val)
    a_f = float(f32(a_val))

    n_ch = rows // (F * 8)
    assert n_ch * F * 8 == rows

    with tile.TileContext(nc) as tc:
        with contextlib.ExitStack() as ctx:
            singles = ctx.enter_context(tc.tile_pool(name="singles", bufs=1))

            eye = singles.tile([128, 128], dt.float32)
            make_identity(nc, eye)
            Wa = singles.tile([128, 128], dt.float32)
            nc.sync.dma_start(out=Wa, in_=bass.AP(tensor=wm_d, offset=0, ap=[[128, 128], [1, 128]]))
            Wb = singles.tile([128, 128], dt.float32)
            nc.sync.dma_start(out=Wb, in_=bass.AP(tensor=wm_d, offset=16384, ap=[[128, 128], [1, 128]]))
            Wo = singles.tile([128, 128], dt.float32)
            nc.sync.dma_start(out=Wo, in_=bass.AP(tensor=wm_d, offset=32768, ap=[[128, 128], [1, 128]]))
            b4A = singles.tile([128, 1], dt.float32)
            nc.sync.dma_start(out=b4A, in_=bass.AP(tensor=b4_d, offset=0, ap=[[1, 128], [0, 1]]))
            b4B = singles.tile([128, 1], dt.float32)
            nc.sync.dma_start(out=b4B, in_=bass.AP(tensor=b4_d, offset=128, ap=[[1, 128], [0, 1]]))
            sp_t = singles.tile([128, 256], dt.float32)
            nc.sync.dma_start(out=sp_t, in_=bass.AP(tensor=sp_d, offset=0, ap=[[0, 128], [1, 256]]))
            ck_t = singles.tile([128, 32], dt.float32)
            nc.sync.dma_start(out=ck_t, in_=bass.AP(tensor=ck_d, offset=0, ap=[[0, 128], [1, 32]]))
            i16_t = singles.tile([128, 16], dt.float32)
            nc.sync.dma_start(out=i16_t, in_=bass.AP(tensor=i16_d, offset=0, ap=[[0, 128], [1, 16]]))
            shv_t = singles.tile([128, 16], dt.int32)
            nc.sync.dma_start(out=shv_t, in_=bass.AP(tensor=shv_d, offset=0, ap=[[0, 128], [1, 16]]))
            quart = singles.tile([128, 1], dt.float32)
            nc.vector.memset(quart, 0.25)

            work = ctx.enter_context(tc.tile_pool(name="work", bufs=2))
            mmp = ctx.enter_context(tc.tile_pool(name="mmp", bufs=1))
            ps_x = ctx.enter_context(tc.tile_pool(name="ps_x", bufs=2, space="PSUM"))
            ps_a1 = ctx.enter_context(tc.tile_pool(name="ps_a1", bufs=1, space="PSUM"))
            ps_a2 = ctx.enter_context(tc.tile_pool(name="ps_a2", bufs=1, space="PSUM"))
            ps_b1 = ctx.enter_context(tc.tile_pool(name="ps_b1", bufs=1, space="PSUM"))
            ps_b2 = ctx.enter_context(tc.tile_pool(name="ps_b2", bufs=1, space="PSUM"))
            ps_t = ctx.enter_context(tc.tile_pool(name="ps_t", bufs=2, space="PSUM"))

            for it in range(n_ch):
                # ---- load x (row layout) and transpose to (b_lo,d) x b_hi ----
                xR = work.tile([128, 4, 8, 16], dt.float32)
                nc.sync.dma_start(
                    out=xR,
                    in_=bass.AP(tensor=x_d, offset=it * 128 * F,
                                ap=[[128, 128], [16384, 4], [16, 8], [1, 16]]),
                )
                psX = ps_x.tile([128, F], dt.float32)
                for j in range(4):
                    nc.tensor.matmul(
                        out=psX[:, j * 128:(j + 1) * 128],
                        lhsT=_ap(xR, [[1, 128]], j * 128),
                        rhs=eye, is_transpose=True, start=True, stop=True,
                    )

                # ---- phase 1: per-(row,d) elementwise in transposed layout ----
                p1 = work.tile([128, F], dt.float32)
                nc.scalar.activation(out=p1, in_=psX, func=Act.Copy, bias=0.0, scale=rl)
                xs = work.tile([128, F], dt.float32)
                nc.vector.scalar_tensor_tensor(out=xs, in0=psX, scalar=rh, in1=p1, op0=Alu.mult, op1=Alu.add)
                t0 = work.tile([128, F], dt.float32)
                nc.scalar.activation(out=t0, in_=xs, func=Act.Copy, bias=MAGIC, scale=1.0)
                r0 = work.tile([128, F], dt.float32)
                nc.scalar.activation(out=r0, in_=t0, func=Act.Copy, bias=-MAGIC, scale=1.0)
                xm1 = p1  # in-place: p1 dead
                nc.scalar.activation(out=xm1, in_=xs, func=Act.Copy, bias=-1.0, scale=1.0)
                t1 = t0  # in-place: t0 dead
                nc.scalar.activation(out=t1, in_=xm1, func=Act.Copy, bias=MAGIC, scale=1.0)
                r1 = work.tile([128, F], dt.float32)
                nc.scalar.activation(out=r1, in_=t1, func=Act.Copy, bias=-MAGIC, scale=1.0)
                e0 = work.tile([128, F], dt.float32)
                nc.vector.tensor_tensor(out=e0, in0=r0, in1=xs, op=Alu.subtract)
                e1 = work.tile([128, F], dt.float32)
                nc.vector.scalar_tensor_tensor(out=e1, in0=r1, scalar=1.0, in1=xs, op0=Alu.add, op1=Alu.subtract)
                s0 = work.tile([128, F], dt.float32)
                nc.scalar.activation(out=s0, in_=e0, func=Act.Square, scale=1.0)
                s1 = xs  # in-place: xs dead
                nc.scalar.activation(out=s1, in_=e1, func=Act.Square, scale=1.0)
                q = e1  # in-place: e1 dead
                nc.vector.tensor_tensor(out=q, in0=s1, in1=s0, op=Alu.subtract)
                dr = work.tile([128, F], dt.float32)
                nc.vector.scalar_tensor_tensor(out=dr, in0=r1, scalar=1.0, in1=r0, op0=Alu.add, op1=Alu.subtract)

                # ---- phase 2: PE matmuls ----
                psA1 = ps_a1.tile([128, F], dt.float32)
                nc.tensor.matmul(out=psA1, lhsT=Wa, rhs=q, start=True, stop=False)
                nc.tensor.matmul(out=psA1, lhsT=Wo, rhs=s0, start=False, stop=True)
                psA2 = ps_a2.tile([128, F], dt.float32)
                nc.tensor.matmul(out=psA2, lhsT=Wb, rhs=q, start=True, stop=False)
                nc.tensor.matmul(out=psA2, lhsT=Wo, rhs=s0, start=False, stop=True)
                psB1 = ps_b1.tile([128, F], dt.float32)
                nc.tensor.matmul(out=psB1, lhsT=Wa, rhs=dr, start=True, stop=False)
                nc.tensor.matmul(out=psB1, lhsT=Wo, rhs=r0, start=False, stop=True)
                psB2 = ps_b2.tile([128, F], dt.float32)
                nc.tensor.matmul(out=psB2, lhsT=Wb, rhs=dr, start=True, stop=False)
                nc.tensor.matmul(out=psB2, lhsT=Wo, rhs=r0, start=False, stop=True)

                # ---- phase 3: parity + D0 evac (per k-half) ----
                D0T = work.tile([128, 2, F], dt.float32)
                oddT = work.tile([128, 2, F], dt.float32)
                hh = work.tile([128, F], dt.float32)
                th = work.tile([128, F], dt.float32)
                dp = work.tile([128, F], dt.float32)
                for half, (psB_h, psA_h, b4_h) in enumerate(
                    ((psB1, psA1, b4A), (psB2, psA2, b4B))
                ):
                    # h = (psB - SC) * 0.25  (exact quarter-integers)
                    nc.vector.scalar_tensor_tensor(
                        out=hh, in0=psB_h, scalar=b4_h, in1=_ap(quart, [[0, F]]),
                        op0=Alu.add, op1=Alu.mult,
                    )
                    nc.scalar.activation(out=th, in_=hh, func=Act.Copy, bias=MAGIC1, scale=1.0)
                    hr = th  # in-place-ish chain
                    nc.scalar.activation(out=hr, in_=th, func=Act.Copy, bias=-MAGIC1, scale=1.0)
                    nc.vector.tensor_tensor(out=dp, in0=hh, in1=hr, op=Alu.subtract)
                    nc.scalar.activation(out=_ap(oddT, [[1, F]], half * F), in_=dp, func=Act.Square, scale=2.0)
                    nc.scalar.activation(out=_ap(D0T, [[1, F]], half * F), in_=psA_h, func=Act.Copy, bias=0.0, scale=1.0)

                # ---- phase 4: PE transposes back to row layout ----
                D0R = work.tile([128, 32, 32], dt.float32)
                oddR = work.tile([128, 32, 32], dt.float32)
                r0R = work.tile([128, 32, 16], dt.float32)
                drR = work.tile([128, 32, 16], dt.float32)
                e0R = work.tile([128, 32, 16], dt.float32)
                xpose = [
                    (_ap(D0T, [[1, F]], 0), _ap(D0R, [[256, 4], [32, 8], [1, 16]], 0), nc.scalar),
                    (_ap(D0T, [[1, F]], F), _ap(D0R, [[256, 4], [32, 8], [1, 16]], 16), nc.scalar),
                    (_ap(oddT, [[1, F]], 0), _ap(oddR, [[256, 4], [32, 8], [1, 16]], 0), nc.vector),
                    (_ap(oddT, [[1, F]], F), _ap(oddR, [[256, 4], [32, 8], [1, 16]], 16), nc.vector),
                    (r0, _ap(r0R, [[128, 4], [16, 8], [1, 16]]), nc.scalar),
                    (dr, _ap(drR, [[128, 4], [16, 8], [1, 16]]), nc.vector),
                    (e0, _ap(e0R, [[128, 4], [16, 8], [1, 16]]), nc.vector),
                ]
                for src, dst, eng in xpose:
                    psT = ps_t.tile([128, F], dt.float32)
                    for j in range(4):
                        nc.tensor.matmul(
                            out=psT[:, j * 128:(j + 1) * 128],
                            lhsT=bass.AP(tensor=src.tensor, offset=src.offset + j * 128, ap=[src.ap[0], [1, 128]]),
                            rhs=eye, is_transpose=True, start=True, stop=True,
                        )
                    if eng is nc.scalar:
                        nc.scalar.activation(out=dst, in_=_ap(psT, [[128, 4], [16, 8], [1, 16]]), func=Act.Copy, bias=0.0, scale=1.0)
                    else:
                        eng.tensor_scalar(out=dst, in0=_ap(psT, [[128, 4], [16, 8], [1, 16]]), scalar1=0.0, scalar2=None, op0=Alu.add)

                # ---- phase 5: penalties, argmin, reconstruction (row layout) ----
                ae0 = work.tile([128, 32, 16], dt.float32)
                nc.scalar.activation(out=ae0, in_=e0R, func=Act.Abs, scale=1.0)
                u5 = ae0  # in-place
                nc.gpsimd.tensor_scalar(out=u5, in0=ae0, scalar1=0.5, scalar2=None, op0=Alu.subtract)

                mm = mmp.tile([128, 32, 16, 16], dt.float32)
                nc.gpsimd.tensor_tensor(
                    out=mm,
                    in0=_ap(u5, [[16, 32], [0, 16], [1, 16]]),
                    in1=_ap(sp_t, [[0, 32], [16, 16], [1, 16]]),
                    op=Alu.mult,
                )
                mx = work.tile([128, 32, 16], dt.float32)
                nc.vector.tensor_reduce(out=mx, in_=mm, axis=AX.X, op=Alu.max)
                mn = work.tile([128, 32, 16], dt.float32)
                nc.vector.tensor_reduce(out=mn, in_=mm, axis=AX.X, op=Alu.min)

                penw = work.tile([128, 32, 32], dt.float32)
                nc.scalar.activation(out=_ap(penw, [[32, 32], [2, 16]]), in_=mx, func=Act.Copy, bias=8.0, scale=-16.0)
                nc.scalar.activation(out=_ap(penw, [[32, 32], [2, 16]], 1), in_=mn, func=Act.Copy, bias=8.0, scale=16.0)
                pen = penw  # in-place
                nc.vector.tensor_tensor(out=pen, in0=penw, in1=oddR, op=Alu.mult)
                Dq = D0R  # in-place
                nc.vector.scalar_tensor_tensor(out=Dq, in0=D0R, scalar=4.0, in1=pen, op0=Alu.mult, op1=Alu.add)

                Dmin = work.tile([128, 32], dt.float32)
                nc.vector.tensor_reduce(out=Dmin, in_=Dq, axis=AX.X, op=Alu.min)
                eq = work.tile([128, 32, 32], dt.float32)
                nc.vector.tensor_tensor(out=eq, in0=Dq, in1=_ap(Dmin, [[1, 32], [0, 32]]), op=Alu.is_equal)
                ovt = pen  # in-place: pen dead
                nc.vector.scalar_tensor_tensor(
                    out=ovt, in0=oddR, scalar=65536.0, in1=_ap(ck_t, [[0, 32], [1, 32]]),
                    op0=Alu.mult, op1=Alu.add,
                )
                m1 = eq  # in-place
                nc.vector.tensor_tensor(out=m1, in0=eq, in1=ovt, op=Alu.mult)
                selneg = work.tile([128, 32], dt.float32)
                nc.vector.tensor_reduce(out=selneg, in_=m1, axis=AX.X, op=Alu.min)

                # decode winner: sel = k*2^18 + odd*2^16 + packed_parity
                sel = selneg  # in-place
                nc.vector.tensor_scalar(out=sel, in0=selneg, scalar1=float(2.0**23), scalar2=None, op0=Alu.add)
                seli = work.tile([128, 32], dt.int32)
                nc.scalar.activation(out=seli, in_=sel, func=Act.Copy, bias=0.0, scale=1.0)
                packi = work.tile([128, 32], dt.int32)
                nc.vector.tensor_scalar(out=packi, in0=seli, scalar1=65535, scalar2=None, op0=Alu.bitwise_and)
                q16i = seli  # in-place
                nc.vector.tensor_scalar(out=q16i, in0=seli, scalar1=16, scalar2=None, op0=Alu.logical_shift_right)
                oddi = q16i  # in-place
                nc.vector.tensor_scalar(out=oddi, in0=q16i, scalar1=1, scalar2=None, op0=Alu.bitwise_and)
                of = Dmin  # in-place: Dmin dead
                nc.scalar.activation(out=of, in_=oddi, func=Act.Copy, bias=0.0, scale=-2.0)

                # unpack parity bits of winner: psel[d] = bit d of packi
                shB = work.tile([128, 32, 16], dt.int32)
                nc.vector.tensor_tensor(
                    out=shB, in0=_ap(packi, [[1, 32], [0, 16]]), in1=_ap(shv_t, [[0, 32], [1, 16]]),
                    op=Alu.arith_shift_right,
                )
                anB = shB  # in-place
                nc.vector.tensor_scalar(out=anB, in0=shB, scalar1=1, scalar2=None, op0=Alu.bitwise_and)
                psel = work.tile([128, 32, 16], dt.float32)
                nc.scalar.activation(out=psel, in_=anB, func=Act.Copy, bias=0.0, scale=1.0)

                # reconstruct winner point + flip
                px = psel  # in-place
                nc.vector.tensor_tensor(out=px, in0=psel, in1=drR, op=Alu.mult)
                XR = r0R  # in-place
                nc.vector.tensor_tensor(out=XR, in0=r0R, in1=px, op=Alu.add)
                esel = e0R  # in-place
                nc.vector.tensor_tensor(out=esel, in0=e0R, in1=px, op=Alu.add)
                ae = px  # in-place
                nc.scalar.activation(out=ae, in_=esel, func=Act.Abs, scale=1.0)
                M16 = work.tile([128, 32], dt.float32)
                nc.vector.tensor_reduce(out=M16, in_=ae, axis=AX.X, op=Alu.max)
                meq = drR  # in-place: drR dead
                nc.vector.tensor_tensor(out=meq, in0=ae, in1=_ap(M16, [[1, 32], [0, 16]]), op=Alu.is_equal)
                m2 = meq  # in-place
                nc.vector.tensor_tensor(out=m2, in0=meq, in1=_ap(i16_t, [[0, 32], [1, 16]]), op=Alu.mult)
                jm = M16  # in-place
                nc.vector.tensor_reduce(out=jm, in_=m2, axis=AX.X, op=Alu.min)
                mask1 = m2  # in-place
                nc.vector.tensor_tensor(
                    out=mask1, in0=_ap(i16_t, [[0, 32], [1, 16]]), in1=_ap(jm, [[1, 32], [0, 16]]), op=Alu.is_equal
                )
                sgn = u5  # in-place: u5 dead
                nc.scalar.activation(out=sgn, in_=esel, func=Act.Sign, scale=1.0)
                corr = mask1  # in-place
                nc.vector.tensor_tensor(out=corr, in0=mask1, in1=sgn, op=Alu.mult)
                u2 = corr  # in-place
                nc.vector.tensor_tensor(out=u2, in0=corr, in1=_ap(of, [[1, 32], [0, 16]]), op=Alu.mult)
                Xf = XR  # in-place
                nc.vector.tensor_tensor(out=Xf, in0=XR, in1=u2, op=Alu.add)
                y_t = Xf  # in-place
                nc.scalar.activation(out=y_t, in_=Xf, func=Act.Copy, bias=0.0, scale=a_f)

                nc.sync.dma_start(
                    out=bass.AP(tensor=y_d, offset=it * 128 * F,
                                ap=[[128, 128], [16384, 4], [16, 8], [1, 16]]),
                    in_=_ap(y_t, [[128, 4], [16, 8], [1, 16]]),
                )
    nc.finalize()
    return nc


def _consts(C, a_val):
    C = np.asarray(C, dtype=f32)
    par = np.mod(C, 2.0).astype(f32)            # (32,16) RM(1,4) codewords
    sp = (1.0 - 2.0 * par).astype(f32)
    SC = C.sum(axis=1).astype(f32)
    pack = (par * (2.0 ** np.arange(16))).sum(axis=1)

    Wa = np.zeros((128, 128), dtype=f32)
    Wb = np.zeros((128, 128), dtype=f32)
    Wo = np.zeros((128, 128), dtype=f32)
    for bl in range(8):
        s = slice(bl * 16, bl * 16 + 16)
        Wa[s, s] = par[0:16].T
        Wb[s, s] = par[16:32].T
        Wo[s, s] = 1.0
    wm = np.concatenate([Wa.reshape(-1), Wb.reshape(-1), Wo.reshape(-1)]).astype(f32)

    b4 = np.empty(256, dtype=f32)
    for p in range(128):
        b4[p] = -SC[p % 16]
        b4[128 + p] = -SC[16 + p % 16]
    sp_blob = sp[0::2].reshape(-1).astype(f32)  # even codewords (16,16)
    ck = (np.arange(32) * 2.0**18 + pack - 2.0**23).astype(f32)
    i16 = (np.arange(16) - 32).astype(f32)
    shv = np.arange(16, dtype=np.int32)
    return {"wm": wm, "b4": b4, "sp": sp_blob, "ck": ck, "i16": i16, "shv": shv}


_CACHE = {}


def _get_nc(rows, a_val):
    key = (rows, a_val)
    if key not in _CACHE:
        _CACHE[key] = _build(rows, a_val)
    return _CACHE[key]


def _in_maps(x, C, a_val):
    B = x.shape[0]
    rows = B // N_CORES
    cst = _consts(C, a_val)
    shards = x.reshape(N_CORES, rows, 16)
    return [{"x": shards[i], **cst} for i in range(N_CORES)]


def kernel(x_in, C_rep, a):
    from concourse.bass_utils import run_bass_kernel_spmd

    x = np.ascontiguousarray(np.asarray(x_in, dtype=np.float32))
    C = np.asarray(C_rep, dtype=np.float32)
    a_val = float(np.asarray(a).reshape(-1)[0])
    B = x.shape[0]
    rows = B // N_CORES
    assert rows * N_CORES == B

    nc = _get_nc(rows, a_val)
    in_maps = _in_maps(x, C, a_val)
    res = run_bass_kernel_spmd(nc, in_maps, core_ids=list(range(N_CORES)))
    y = np.concatenate([res.results[i]["y"] for i in range(N_CORES)], axis=0)
    return y.astype(np.float32)


if __name__ == "__main__":
    rng = np.random.default_rng(0)
    x = rng.standard_normal((262144, 16), dtype=np.float32)
    import itertools
    _RM = np.array([
        [1,1,1,1,0,1,0,1,1,0,0,1,0,0,0,0],
        [0,1,1,1,1,0,1,0,1,1,0,0,1,0,0,0],
        [0,0,1,1,1,1,0,1,0,1,1,0,0,1,0,0],
        [0,0,0,1,1,1,1,0,1,0,1,1,0,0,1,0],
        [1,1,1,1,1,1,1,1,1,1,1,1,1,1,1,1]], dtype=np.float64)
    bits = np.array(list(itertools.product([0, 1], repeat=5)), dtype=np.float64)
    C = (bits @ _RM).astype(np.float32)
    a = np.array([0.59460354], dtype=np.float32)
    y = kernel(x, C, a)
    print("ok", y.shape, y.dtype)


# revision 17
# speedup vs baseline: 6.1707x; 2.4861x over previous
"""Barnes-Wall (BW16) lattice quantizer for Trainium2, 8-core data-parallel.

Algorithm (validated bit-exact vs the jax reference in numpy):
  The 32 codewords C differ per-dimension only through parity (their mod-2
  patterns are the RM(1,4) codewords).  So per row only TWO rounding chains
  are needed: r0 = nearest even integer vector to x' = x/a, r1+1 = nearest odd
  integer vector, with offsets e0 = r0-x', e1 = (r1+1)-x' (e1-e0 = dr = +-1
  exactly).  Every per-candidate quantity is then a parity-masked sum:
    D0_k = sum(s0) + sum(par_k * q)       (q = e1^2-e0^2, s0 = e0^2)
    P2_k = sum(r0) + sum(par_k * dr) - SC_k
  computed on the idle PE as block-diagonal matmuls in a transposed layout
  (partitions = 8 batch-sub x 16 dims).  The flip penalty needs
  M_k = max_d(par ? 1-|e0| : |e0|) = 0.5 + max_d(sp_k * u5), u5 = |e0|-0.5,
  done per complement-pair (even codewords only, max and min reduces) in row
  layout after PE transposes.  Winner selection packs (k, odd, parity-bits)
  into one fp32 constant per candidate so a single min-reduce recovers
  everything; parity bits are unpacked with exact fp32 mod arithmetic.
"""
import sys

sys.path.insert(0, "/opt/trn_rl_repo")
import contextlib

import numpy as np

import concourse.bass as bass
import concourse.bacc as bacc
import concourse.mybir as mybir
import concourse.tile as tile
from concourse.masks import make_identity

f32 = np.float32
MAGIC = float(f32(1.5 * 2.0**24))   # round-to-even-integer magic
MAGIC1 = float(f32(1.5 * 2.0**23))  # round-to-integer magic (parity)

dt = mybir.dt
Alu = mybir.AluOpType
Act = mybir.ActivationFunctionType
AX = mybir.AxisListType

N_CORES = 8
F = 512  # b_hi columns per chunk


def _div_consts(a_val):
    r64 = 1.0 / np.float64(f32(a_val))
    rh = f32(r64)
    rl = f32(r64 - np.float64(rh))
    return float(rh), float(rl)


def _ap(t, pattern, off=0):
    return bass.AP(tensor=t.tensor, offset=t.offset + off, ap=[t.ap[0]] + pattern)


def _build(rows, a_val):
    nc = bacc.Bacc("TRN2", target_bir_lowering=False)
    x_d = nc.dram_tensor("x", [rows, 16], dt.float32, kind="ExternalInput")
    wm_d = nc.dram_tensor("wm", [4 * 16384], dt.float32, kind="ExternalInput")
    b4_d = nc.dram_tensor("b4", [256], dt.float32, kind="ExternalInput")
    ck_d = nc.dram_tensor("ck", [32], dt.float32, kind="ExternalInput")
    i16_d = nc.dram_tensor("i16", [16], dt.float32, kind="ExternalInput")
    shv_d = nc.dram_tensor("shv", [16], dt.int32, kind="ExternalInput")
    y_d = nc.dram_tensor("y", [rows, 16], dt.float32, kind="ExternalOutput")

    rh, rl = _div_consts(a_val)
    a_f = float(f32(a_val))

    n_ch = rows // (F * 8)
    assert n_ch * F * 8 == rows

    with tile.TileContext(nc) as tc:
        with contextlib.ExitStack() as ctx:
            singles = ctx.enter_context(tc.tile_pool(name="singles", bufs=1))

            eye = singles.tile([128, 128], dt.float32)
            make_identity(nc, eye)
            Wa = singles.tile([128, 128], dt.float32)
            nc.sync.dma_start(out=Wa, in_=bass.AP(tensor=wm_d, offset=0, ap=[[128, 128], [1, 128]]))
            Wb = singles.tile([128, 128], dt.float32)
            nc.sync.dma_start(out=Wb, in_=bass.AP(tensor=wm_d, offset=16384, ap=[[128, 128], [1, 128]]))
            Wo = singles.tile([128, 128], dt.float32)
            nc.sync.dma_start(out=Wo, in_=bass.AP(tensor=wm_d, offset=32768, ap=[[128, 128], [1, 128]]))
            PR = singles.tile([128, 128], dt.float32)
            nc.sync.dma_start(out=PR, in_=bass.AP(tensor=wm_d, offset=49152, ap=[[128, 128], [1, 128]]))
            b4A = singles.tile([128, 1], dt.float32)
            nc.sync.dma_start(out=b4A, in_=bass.AP(tensor=b4_d, offset=0, ap=[[1, 128], [0, 1]]))
            b4B = singles.tile([128, 1], dt.float32)
            nc.sync.dma_start(out=b4B, in_=bass.AP(tensor=b4_d, offset=128, ap=[[1, 128], [0, 1]]))
            ck_t = singles.tile([128, 32], dt.float32)
            nc.sync.dma_start(out=ck_t, in_=bass.AP(tensor=ck_d, offset=0, ap=[[0, 128], [1, 32]]))
            i16_t = singles.tile([128, 16], dt.float32)
            nc.sync.dma_start(out=i16_t, in_=bass.AP(tensor=i16_d, offset=0, ap=[[0, 128], [1, 16]]))
            shv_t = singles.tile([128, 16], dt.int32)
            nc.sync.dma_start(out=shv_t, in_=bass.AP(tensor=shv_d, offset=0, ap=[[0, 128], [1, 16]]))

            work = ctx.enter_context(tc.tile_pool(name="work", bufs=2))
            ps_x = ctx.enter_context(tc.tile_pool(name="ps_x", bufs=2, space="PSUM"))
            ps_a1 = ctx.enter_context(tc.tile_pool(name="ps_a1", bufs=1, space="PSUM"))
            ps_a2 = ctx.enter_context(tc.tile_pool(name="ps_a2", bufs=1, space="PSUM"))
            ps_b1 = ctx.enter_context(tc.tile_pool(name="ps_b1", bufs=1, space="PSUM"))
            ps_b2 = ctx.enter_context(tc.tile_pool(name="ps_b2", bufs=1, space="PSUM"))
            ps_t = ctx.enter_context(tc.tile_pool(name="ps_t", bufs=2, space="PSUM"))

            for it in range(n_ch):
                # ---- load x (row layout) and transpose to (b_lo,d) x b_hi ----
                xR = work.tile([128, 4, 8, 16], dt.float32)
                nc.sync.dma_start(
                    out=xR,
                    in_=bass.AP(tensor=x_d, offset=it * 128 * F,
                                ap=[[128, 128], [16384, 4], [16, 8], [1, 16]]),
                )
                psX = ps_x.tile([128, F], dt.float32)
                for j in range(4):
                    nc.tensor.matmul(
                        out=psX[:, j * 128:(j + 1) * 128],
                        lhsT=_ap(xR, [[1, 128]], j * 128),
                        rhs=eye, is_transpose=True, start=True, stop=True,
                    )

                # ---- phase 1: per-(row,d) elementwise in transposed layout ----
                p1 = work.tile([128, F], dt.float32)
                nc.scalar.activation(out=p1, in_=psX, func=Act.Copy, bias=0.0, scale=rl)
                xs = work.tile([128, F], dt.float32)
                nc.vector.scalar_tensor_tensor(out=xs, in0=psX, scalar=rh, in1=p1, op0=Alu.mult, op1=Alu.add)
                t0 = work.tile([128, F], dt.float32)
                nc.scalar.activation(out=t0, in_=xs, func=Act.Copy, bias=MAGIC, scale=1.0)
                r0 = work.tile([128, F], dt.float32)
                nc.scalar.activation(out=r0, in_=t0, func=Act.Copy, bias=-MAGIC, scale=1.0)
                xm1 = p1  # in-place: p1 dead
                nc.scalar.activation(out=xm1, in_=xs, func=Act.Copy, bias=-1.0, scale=1.0)
                t1 = t0  # in-place: t0 dead
                nc.scalar.activation(out=t1, in_=xm1, func=Act.Copy, bias=MAGIC, scale=1.0)
                r1 = work.tile([128, F], dt.float32)
                nc.scalar.activation(out=r1, in_=t1, func=Act.Copy, bias=-MAGIC, scale=1.0)
                e0 = work.tile([128, F], dt.float32)
                nc.vector.tensor_tensor(out=e0, in0=r0, in1=xs, op=Alu.subtract)
                e1 = work.tile([128, F], dt.float32)
                nc.vector.scalar_tensor_tensor(out=e1, in0=r1, scalar=1.0, in1=xs, op0=Alu.add, op1=Alu.subtract)
                s0 = work.tile([128, F], dt.float32)
                nc.scalar.activation(out=s0, in_=e0, func=Act.Square, scale=1.0)
                s1 = xs  # in-place: xs dead
                nc.scalar.activation(out=s1, in_=e1, func=Act.Square, scale=1.0)
                q = e1  # in-place: e1 dead
                nc.vector.tensor_tensor(out=q, in0=s1, in1=s0, op=Alu.subtract)
                dr = work.tile([128, F], dt.float32)
                nc.vector.scalar_tensor_tensor(out=dr, in0=r1, scalar=1.0, in1=r0, op0=Alu.add, op1=Alu.subtract)

                # ---- phase 2: PE matmuls ----
                psA1 = ps_a1.tile([128, F], dt.float32)
                nc.tensor.matmul(out=psA1, lhsT=Wa, rhs=q, start=True, stop=False)
                nc.tensor.matmul(out=psA1, lhsT=Wo, rhs=s0, start=False, stop=True)
                psA2 = ps_a2.tile([128, F], dt.float32)
                nc.tensor.matmul(out=psA2, lhsT=Wb, rhs=q, start=True, stop=False)
                nc.tensor.matmul(out=psA2, lhsT=Wo, rhs=s0, start=False, stop=True)
                psB1 = ps_b1.tile([128, F], dt.float32)
                nc.tensor.matmul(out=psB1, lhsT=Wa, rhs=dr, start=True, stop=False)
                nc.tensor.matmul(out=psB1, lhsT=Wo, rhs=r0, start=False, stop=True)
                psB2 = ps_b2.tile([128, F], dt.float32)
                nc.tensor.matmul(out=psB2, lhsT=Wb, rhs=dr, start=True, stop=False)
                nc.tensor.matmul(out=psB2, lhsT=Wo, rhs=r0, start=False, stop=True)

                # ---- phase 3: parity + D0 evac (per k-half) ----
                D0T = work.tile([128, 2, F], dt.float32)
                oddT = work.tile([128, 2, F], dt.float32)
                hh = work.tile([128, F], dt.float32)
                th = work.tile([128, F], dt.float32)
                dp = work.tile([128, F], dt.float32)
                for half, (psB_h, psA_h, b4_h) in enumerate(
                    ((psB1, psA1, b4A), (psB2, psA2, b4B))
                ):
                    # h = psB * 0.25 - SC/4  (exact quarter-integers)
                    nc.scalar.activation(out=hh, in_=psB_h, func=Act.Identity, bias=b4_h, scale=0.25)
                    nc.scalar.activation(out=th, in_=hh, func=Act.Copy, bias=MAGIC1, scale=1.0)
                    hr = th  # in-place-ish chain
                    nc.scalar.activation(out=hr, in_=th, func=Act.Copy, bias=-MAGIC1, scale=1.0)
                    nc.vector.tensor_tensor(out=dp, in0=hh, in1=hr, op=Alu.subtract)
                    nc.scalar.activation(out=_ap(oddT, [[1, F]], half * F), in_=dp, func=Act.Square, scale=2.0)
                    nc.scalar.activation(out=_ap(D0T, [[1, F]], half * F), in_=psA_h, func=Act.Copy, bias=0.0, scale=1.0)

                # ---- phase 4: PE transposes back to row layout ----
                D0R = work.tile([128, 32, 32], dt.float32)
                oddR = work.tile([128, 32, 32], dt.float32)
                r0R = work.tile([128, 32, 16], dt.float32)
                drR = work.tile([128, 32, 16], dt.float32)
                e0R = work.tile([128, 32, 16], dt.float32)
                e0Rp = work.tile([128, 32, 16], dt.float32)  # label-permuted d-order
                xpose = [
                    (_ap(D0T, [[1, F]], 0), _ap(D0R, [[256, 4], [32, 8], [1, 16]], 0), eye),
                    (_ap(D0T, [[1, F]], F), _ap(D0R, [[256, 4], [32, 8], [1, 16]], 16), eye),
                    (_ap(oddT, [[1, F]], 0), _ap(oddR, [[256, 4], [32, 8], [1, 16]], 0), eye),
                    (_ap(oddT, [[1, F]], F), _ap(oddR, [[256, 4], [32, 8], [1, 16]], 16), eye),
                    (r0, _ap(r0R, [[128, 4], [16, 8], [1, 16]]), eye),
                    (dr, _ap(drR, [[128, 4], [16, 8], [1, 16]]), eye),
                    (e0, _ap(e0R, [[128, 4], [16, 8], [1, 16]]), eye),
                    (e0, _ap(e0Rp, [[128, 4], [16, 8], [1, 16]]), PR),
                ]
                for src, dst, rhs_t in xpose:
                    psT = ps_t.tile([128, F], dt.float32)
                    for j in range(4):
                        nc.tensor.matmul(
                            out=psT[:, j * 128:(j + 1) * 128],
                            lhsT=bass.AP(tensor=src.tensor, offset=src.offset + j * 128, ap=[src.ap[0], [1, 128]]),
                            rhs=rhs_t, is_transpose=True, start=True, stop=True,
                        )
                    nc.scalar.activation(out=dst, in_=_ap(psT, [[128, 4], [16, 8], [1, 16]]), func=Act.Copy, bias=0.0, scale=1.0)

                # ---- phase 5: penalties via max-Hadamard butterfly over label bits ----
                ae0p = work.tile([128, 32, 16], dt.float32)
                nc.scalar.activation(out=ae0p, in_=e0Rp, func=Act.Abs, scale=1.0)
                # state S[r, pq, slot]: pq=0 holds P=max-side (u5), pq=1 holds Q (-u5)
                Sa = work.tile([128, 32, 2, 16], dt.float32)
                Sb = work.tile([128, 32, 2, 16], dt.float32)
                nc.vector.tensor_scalar(out=_ap(Sa, [[32, 32], [1, 16]], 0), in0=ae0p,
                                        scalar1=0.5, scalar2=None, op0=Alu.subtract)
                nc.vector.tensor_scalar(out=_ap(Sa, [[32, 32], [1, 16]], 16), in0=ae0p,
                                        scalar1=-1.0, scalar2=0.5, op0=Alu.mult, op1=Alu.add)
                cur, nxt = Sa, Sb
                for t in range(4):
                    sg, sf = 1 << t, 1 << (3 - t)  # sigma count, suffix count (= pair offset)
                    pat = [[16, 64]] + ([[16 >> t, sg]] if sg > 1 else []) + ([[1, sf]] if sf > 1 else [])
                    # + branch (new bit 0): P'=max(PA,PB), Q'=max(QA,QB) in one op
                    nc.vector.tensor_tensor(out=_ap(nxt, pat, 0), in0=_ap(cur, pat, 0),
                                            in1=_ap(cur, pat, sf), op=Alu.max)
                    # - branch (new bit 1): P'=max(PA,QB), Q'=max(QA,PB) -> in1 pq-swapped
                    if t in (0, 3):
                        swpat = [[32, 32], [-16, 2]] + ([[16 >> t, sg]] if sg > 1 else []) + ([[1, sf]] if sf > 1 else [])
                        nc.vector.tensor_tensor(out=_ap(nxt, pat, sf), in0=_ap(cur, pat, 0),
                                                in1=_ap(cur, swpat, 16 + sf), op=Alu.max)
                    else:
                        p2 = [[32, 32], [16 >> t, sg], [1, sf]]
                        nc.vector.tensor_tensor(out=_ap(nxt, p2, sf), in0=_ap(cur, p2, 0),
                                                in1=_ap(cur, p2, 16 + sf), op=Alu.max)
                        nc.vector.tensor_tensor(out=_ap(nxt, p2, 16 + sf), in0=_ap(cur, p2, 16),
                                                in1=_ap(cur, p2, sf), op=Alu.max)
                    cur, nxt = nxt, cur
                # pen weights: even k'=2m: 8-16*P[m]; odd k'=2m+1: 8+16*mn = 8-16*Q[m]
                penw = work.tile([128, 32, 32], dt.float32)
                nc.scalar.activation(out=_ap(penw, [[32, 32], [1, 2], [2, 16]]),
                                     in_=_ap(cur, [[32, 32], [16, 2], [1, 16]]),
                                     func=Act.Copy, bias=8.0, scale=-16.0)
                pen = penw  # in-place
                nc.vector.tensor_tensor(out=pen, in0=penw, in1=oddR, op=Alu.mult)
                Dq = D0R  # in-place
                nc.vector.scalar_tensor_tensor(out=Dq, in0=D0R, scalar=4.0, in1=pen, op0=Alu.mult, op1=Alu.add)

                Dmin = work.tile([128, 32], dt.float32)
                nc.vector.tensor_reduce(out=Dmin, in_=Dq, axis=AX.X, op=Alu.min)
                eq = work.tile([128, 32, 32], dt.float32)
                nc.vector.tensor_tensor(out=eq, in0=Dq, in1=_ap(Dmin, [[1, 32], [0, 32]]), op=Alu.is_equal)
                ovt = pen  # in-place: pen dead
                nc.vector.scalar_tensor_tensor(
                    out=ovt, in0=oddR, scalar=65536.0, in1=_ap(ck_t, [[0, 32], [1, 32]]),
                    op0=Alu.mult, op1=Alu.add,
                )
                m1 = eq  # in-place
                nc.vector.tensor_tensor(out=m1, in0=eq, in1=ovt, op=Alu.mult)
                selneg = work.tile([128, 32], dt.float32)
                nc.vector.tensor_reduce(out=selneg, in_=m1, axis=AX.X, op=Alu.min)

                # decode winner: sel = k*2^18 + odd*2^16 + packed_parity
                sel = selneg  # in-place
                nc.vector.tensor_scalar(out=sel, in0=selneg, scalar1=float(2.0**23), scalar2=None, op0=Alu.add)
                seli = work.tile([128, 32], dt.int32)
                nc.scalar.activation(out=seli, in_=sel, func=Act.Copy, bias=0.0, scale=1.0)
                packi = work.tile([128, 32], dt.int32)
                nc.vector.tensor_scalar(out=packi, in0=seli, scalar1=65535, scalar2=None, op0=Alu.bitwise_and)
                q16i = seli  # in-place
                nc.vector.tensor_scalar(out=q16i, in0=seli, scalar1=16, scalar2=None, op0=Alu.logical_shift_right)
                oddi = q16i  # in-place
                nc.vector.tensor_scalar(out=oddi, in0=q16i, scalar1=1, scalar2=None, op0=Alu.bitwise_and)
                of = Dmin  # in-place: Dmin dead
                nc.scalar.activation(out=of, in_=oddi, func=Act.Copy, bias=0.0, scale=-2.0)

                # unpack parity bits of winner: psel[d] = bit d of packi
                shB = work.tile([128, 32, 16], dt.int32)
                nc.vector.tensor_tensor(
                    out=shB, in0=_ap(packi, [[1, 32], [0, 16]]), in1=_ap(shv_t, [[0, 32], [1, 16]]),
                    op=Alu.arith_shift_right,
                )
                anB = shB  # in-place
                nc.vector.tensor_scalar(out=anB, in0=shB, scalar1=1, scalar2=None, op0=Alu.bitwise_and)
                psel = work.tile([128, 32, 16], dt.float32)
                nc.scalar.activation(out=psel, in_=anB, func=Act.Copy, bias=0.0, scale=1.0)

                # reconstruct winner point + flip
                px = psel  # in-place
                nc.vector.tensor_tensor(out=px, in0=psel, in1=drR, op=Alu.mult)
                XR = r0R  # in-place
                nc.vector.tensor_tensor(out=XR, in0=r0R, in1=px, op=Alu.add)
                esel = e0R  # in-place
                nc.vector.tensor_tensor(out=esel, in0=e0R, in1=px, op=Alu.add)
                ae = px  # in-place
                nc.scalar.activation(out=ae, in_=esel, func=Act.Abs, scale=1.0)
                M16 = work.tile([128, 32], dt.float32)
                nc.vector.tensor_reduce(out=M16, in_=ae, axis=AX.X, op=Alu.max)
                meq = drR  # in-place: drR dead
                nc.vector.tensor_tensor(out=meq, in0=ae, in1=_ap(M16, [[1, 32], [0, 16]]), op=Alu.is_equal)
                m2 = meq  # in-place
                nc.vector.tensor_tensor(out=m2, in0=meq, in1=_ap(i16_t, [[0, 32], [1, 16]]), op=Alu.mult)
                jm = M16  # in-place
                nc.vector.tensor_reduce(out=jm, in_=m2, axis=AX.X, op=Alu.min)
                mask1 = m2  # in-place
                nc.vector.tensor_tensor(
                    out=mask1, in0=_ap(i16_t, [[0, 32], [1, 16]]), in1=_ap(jm, [[1, 32], [0, 16]]), op=Alu.is_equal
                )
                sgn = ae0p  # in-place: ae0p dead
                nc.scalar.activation(out=sgn, in_=esel, func=Act.Sign, scale=1.0)
                corr = mask1  # in-place
                nc.vector.tensor_tensor(out=corr, in0=mask1, in1=sgn, op=Alu.mult)
                u2 = corr  # in-place
                nc.vector.tensor_tensor(out=u2, in0=corr, in1=_ap(of, [[1, 32], [0, 16]]), op=Alu.mult)
                Xf = XR  # in-place
                nc.vector.tensor_tensor(out=Xf, in0=XR, in1=u2, op=Alu.add)
                y_t = Xf  # in-place
                nc.scalar.activation(out=y_t, in_=Xf, func=Act.Copy, bias=0.0, scale=a_f)

                nc.sync.dma_start(
                    out=bass.AP(tensor=y_d, offset=it * 128 * F,
                                ap=[[128, 128], [16384, 4], [16, 8], [1, 16]]),
                    in_=_ap(y_t, [[128, 4], [16, 8], [1, 16]]),
                )
    nc.finalize()
    return nc


def _consts(C, a_val):
    C = np.asarray(C, dtype=f32)
    par = np.mod(C, 2.0).astype(f32)            # (32,16) RM(1,4) codewords
    SC = C.sum(axis=1).astype(f32)
    pack = (par * (2.0 ** np.arange(16))).sum(axis=1)

    # labels: lam(d) = parity pattern bits of the 4 linear generators at dim d.
    # The generators are recovered from the codebook parities: even codewords
    # par[2j] = bits(j) @ G[0:4]; generator row t = par of the codeword with
    # b_t = 1 only, i.e. j = 2^(3-t) -> k = 2j... derive directly:
    # par[2*j] for j = 8,4,2,1 are the 4 generator rows (b0..b3 single-bit).
    G4 = np.stack([par[2 * 8], par[2 * 4], par[2 * 2], par[2 * 1]]).astype(np.int64)
    lv = (G4.T @ (2 ** np.arange(4))).astype(np.int64)  # label per dim
    assert len(set(lv.tolist())) == 16

    # candidate reorder: k' = 2m + pq, m = butterfly slot
    order = []
    for m in range(16):
        b = [(m >> t) & 1 for t in range(4)]
        k_even = 16 * b[0] + 8 * b[1] + 4 * b[2] + 2 * b[3]
        order += [k_even, k_even + 1]
    order = np.array(order)
    par_o = par[order]
    SC_o = SC[order]
    pack_o = pack[order]

    Wa = np.zeros((128, 128), dtype=f32)
    Wb = np.zeros((128, 128), dtype=f32)
    Wo = np.zeros((128, 128), dtype=f32)
    PR = np.zeros((128, 128), dtype=f32)
    for bl in range(8):
        s = slice(bl * 16, bl * 16 + 16)
        Wa[s, s] = par_o[0:16].T
        Wb[s, s] = par_o[16:32].T
        Wo[s, s] = 1.0
        for d in range(16):
            PR[bl * 16 + d, bl * 16 + lv[d]] = 1.0
    wm = np.concatenate([Wa.reshape(-1), Wb.reshape(-1), Wo.reshape(-1), PR.reshape(-1)]).astype(f32)

    b4 = np.empty(256, dtype=f32)
    for p in range(128):
        b4[p] = -SC_o[p % 16] * 0.25
        b4[128 + p] = -SC_o[16 + p % 16] * 0.25
    ck = (order * 2.0**18 + pack_o - 2.0**23).astype(f32)
    i16 = (np.arange(16) - 32).astype(f32)
    shv = np.arange(16, dtype=np.int32)
    return {"wm": wm, "b4": b4, "ck": ck, "i16": i16, "shv": shv}


_CACHE = {}


def _get_nc(rows, a_val):
    key = (rows, a_val)
    if key not in _CACHE:
        _CACHE[key] = _build(rows, a_val)
    return _CACHE[key]


def _in_maps(x, C, a_val):
    B = x.shape[0]
    rows = B // N_CORES
    cst = _consts(C, a_val)
    shards = x.reshape(N_CORES, rows, 16)
    return [{"x": shards[i], **cst} for i in range(N_CORES)]


def kernel(x_in, C_rep, a):
    from concourse.bass_utils import run_bass_kernel_spmd

    x = np.ascontiguousarray(np.asarray(x_in, dtype=np.float32))
    C = np.asarray(C_rep, dtype=np.float32)
    a_val = float(np.asarray(a).reshape(-1)[0])
    B = x.shape[0]
    rows = B // N_CORES
    assert rows * N_CORES == B

    nc = _get_nc(rows, a_val)
    in_maps = _in_maps(x, C, a_val)
    res = run_bass_kernel_spmd(nc, in_maps, core_ids=list(range(N_CORES)))
    y = np.concatenate([res.results[i]["y"] for i in range(N_CORES)], axis=0)
    return y.astype(np.float32)


if __name__ == "__main__":
    rng = np.random.default_rng(0)
    x = rng.standard_normal((262144, 16), dtype=np.float32)
    import itertools
    _RM = np.array([
        [1,1,1,1,0,1,0,1,1,0,0,1,0,0,0,0],
        [0,1,1,1,1,0,1,0,1,1,0,0,1,0,0,0],
        [0,0,1,1,1,1,0,1,0,1,1,0,0,1,0,0],
        [0,0,0,1,1,1,1,0,1,0,1,1,0,0,1,0],
        [1,1,1,1,1,1,1,1,1,1,1,1,1,1,1,1]], dtype=np.float64)
    bits = np.array(list(itertools.product([0, 1], repeat=5)), dtype=np.float64)
    C = (bits @ _RM).astype(np.float32)
    a = np.array([0.59460354], dtype=np.float32)
    y = kernel(x, C, a)
    print("ok", y.shape, y.dtype)
